# revision 53
# baseline (speedup 1.0000x reference)
"""Trainium2 Bass kernel for adaptive_high_order_residual_v2 (ORDER=2 masked
sign-binarization, per-row stats).

Full-input contract: kernel(x, mask) takes the complete (4096, 11008) arrays,
shards rows across 8 NeuronCores (512 rows each; per-row reductions make this
embarrassingly parallel), runs one SPMD Bass program, and concatenates the
per-core outputs.

Math per row (ORDER = 2, exact restructuring of the reference):
  t    = x * m                      (masked input)
  mean1 = sum(t)/cnt ; var1 = sum(t^2)/cnt - mean1^2 ; s1 = sqrt(var1 * 2/pi)
  b1   = sign(x - mean1)            (valid entries only; invalid masked later)
  q    = (|x - mean1| - s1) * b1*m  (== residual_2 of the reference: d - s1*b1)
  mean2 = sum(q)/cnt ; var2 = sum(q^2)/cnt - mean2^2 ; s2 = sqrt(var2 * 2/pi)
  b2   = sign(q - mean2)
  out  = ((mean1 + mean2) + s1*b1 + s2*b2) * m

Engine split per 128x2752 chunk:
  ACT: mask cast (+cnt accum), square(T) (+r2), Sign->b1, Abs->|d|,
       square(q) (+sum q^2), Sign->b2
  DVE: T=x*m (+r1 accum), b1m=b1*m16 (bf16 2x), q=(|d|-s1)*b1m (+sum q),
       p1=s1*b1m+K (2x), p2=s2*b2+p1, out=p2*m
"""

import sys

import numpy as np

sys.path.insert(0, "/opt/trn_rl_repo")

R = 512          # rows per core
N = 11008        # columns
P = 128          # SBUF partitions per row-block
NBLK = R // P    # 4 blocks per core
CW = 2752        # column chunk width
NCH = N // CW    # 4 chunks per block
NCORES = 8
C2 = 0.6366197723675814  # 2/pi

_CACHE = {}


def _build_program():
    import concourse.bacc as bacc
    import concourse.mybir as mybir
    from concourse.tile import TileContext

    F32 = mybir.dt.float32
    BF16 = mybir.dt.bfloat16
    U8 = mybir.dt.uint8
    Alu = mybir.AluOpType
    Act = mybir.ActivationFunctionType

    nc = bacc.Bacc()
    x = nc.dram_tensor("x", [R, N], F32, kind="ExternalInput")
    mk = nc.dram_tensor("mask", [R, N], U8, kind="ExternalInput")
    out = nc.dram_tensor("out", [R, N], F32, kind="ExternalOutput")

    with TileContext(nc) as tc:
        with (
            tc.tile_pool(name="xq", bufs=6) as xq_pool,    # x tile, later holds q
            tc.tile_pool(name="m8", bufs=2) as m8_pool,    # raw u8 mask (cast only)
            tc.tile_pool(name="m16", bufs=5) as m16_pool,  # bf16 mask
            tc.tile_pool(name="bmp", bufs=5) as bm_pool,   # masked sign1 (bf16)
            tc.tile_pool(name="b2p", bufs=1) as b2_pool,   # sign2 + ACT garbage
            tc.tile_pool(name="w", bufs=4) as w_pool,      # f32 rotating work
            tc.tile_pool(name="tp", bufs=3) as t_pool,     # T tiles (stage 1)
            tc.tile_pool(name="sc", bufs=2) as sc_pool,    # scalars + accums
        ):
            for b in range(NBLK):
                r0 = b * P

                xt = [
                    xq_pool.tile([P, CW], F32, name=f"xt{b}_{c}", tag="xq")
                    for c in range(NCH)
                ]
                mt = [
                    m8_pool.tile([P, CW], U8, name=f"mt{b}_{c}", tag="m8")
                    for c in range(NCH)
                ]
                m16 = [
                    m16_pool.tile([P, CW], BF16, name=f"m16_{b}_{c}", tag="m16")
                    for c in range(NCH)
                ]
                bm = [
                    bm_pool.tile([P, CW], BF16, name=f"bm{b}_{c}", tag="bm")
                    for c in range(NCH)
                ]
                # accumulators, chunk-major interleave: col = c*nq + q so the
                # pairwise tree reduce uses contiguous 2-D slices
                acc1 = sc_pool.tile([P, 3 * 2 * NCH], F32, name=f"acc1_{b}", tag="acc1")
                acc2 = sc_pool.tile([P, 2 * (NCH + 1)], F32, name=f"acc2_{b}", tag="acc2")
                st1 = sc_pool.tile([P, 3], F32, name=f"st1_{b}", tag="st1")
                st2 = sc_pool.tile([P, 2], F32, name=f"st2_{b}", tag="st2")
                red1 = sc_pool.tile([P, 18], F32, name=f"red1_{b}", tag="red1")
                red2 = sc_pool.tile([P, 6], F32, name=f"red2_{b}", tag="red2")
                sv = sc_pool.tile([P, 24], F32, name=f"sv_{b}", tag="sv")

                def col(t, i):
                    return t[:, i : i + 1]

                # piece lists: (chunk, col offset, width). The first chunk of
                # each stage is split in half so the consuming engine starts
                # ~1.2us sooner after a stage boundary; block 0 also splits
                # the very first DMA for a faster ramp.
                H = CW // 2

                def pieces_for(split_first, split_last=False):
                    ps = []
                    for c in range(NCH):
                        if (c == 0 and split_first) or (
                            c == NCH - 1 and split_last
                        ):
                            ps.append((c, 0, H))
                            ps.append((c, H, H))
                        else:
                            ps.append((c, 0, CW))
                    return ps

                s1_pieces = pieces_for(False)
                s2_pieces = pieces_for(False)
                s3_pieces = pieces_for(False, split_last=(b == NBLK - 1))

                def reduce_cols(dst, acc, nq, npieces, red):
                    # sum piece-major accum columns: col = p*nq + q
                    if npieces == 8:
                        nc.vector.tensor_add(
                            red[:, 0 : 4 * nq], acc[:, 0 : 4 * nq],
                            acc[:, 4 * nq : 8 * nq],
                        )
                        nc.vector.tensor_add(
                            red[:, 4 * nq : 6 * nq], red[:, 0 : 2 * nq],
                            red[:, 2 * nq : 4 * nq],
                        )
                        nc.vector.tensor_add(
                            dst, red[:, 4 * nq : 5 * nq], red[:, 5 * nq : 6 * nq]
                        )
                    elif npieces == 4:
                        nc.vector.tensor_add(
                            red[:, 0 : 2 * nq], acc[:, 0 : 2 * nq],
                            acc[:, 2 * nq : 4 * nq],
                        )
                        nc.vector.tensor_add(dst, red[:, 0:nq], red[:, nq : 2 * nq])
                    elif npieces == 5:
                        nc.vector.tensor_add(
                            red[:, 0 : 2 * nq], acc[:, 0 : 2 * nq],
                            acc[:, 2 * nq : 4 * nq],
                        )
                        nc.vector.tensor_add(
                            red[:, 2 * nq : 3 * nq], red[:, 0:nq], red[:, nq : 2 * nq]
                        )
                        nc.vector.tensor_add(
                            dst, red[:, 2 * nq : 3 * nq], acc[:, 4 * nq : 5 * nq]
                        )
                    else:
                        raise AssertionError(npieces)

                # ------------- stage 1: masked first-order stats -------------
                for i, (c, o, wd) in enumerate(s1_pieces):
                    if o == 0:
                        nc.sync.dma_start(
                            xt[c][:, 0:wd], x[r0 : r0 + P, c * CW : c * CW + wd]
                        )
                        nc.sync.dma_start(
                            mt[c][:, 0:wd], mk[r0 : r0 + P, c * CW : c * CW + wd]
                        )
                    else:
                        nc.sync.dma_start(
                            xt[c][:, o : o + wd],
                            x[r0 : r0 + P, c * CW + o : c * CW + o + wd],
                        )
                        nc.sync.dma_start(
                            mt[c][:, o : o + wd],
                            mk[r0 : r0 + P, c * CW + o : c * CW + o + wd],
                        )
                    # mask cast to bf16 + cnt partial, on DVE at 2x
                    # (single-src tensor_scalar); lightens the ACT queue that
                    # gates every stage boundary
                    nc.vector.tensor_scalar(
                        m16[c][:, o : o + wd],
                        mt[c][:, o : o + wd],
                        1.0,
                        None,
                        Alu.mult,
                        Alu.add,
                        accum_out=col(acc1, i * 3 + 0),
                    )
                    # T = x*m + r1 partial
                    tt = t_pool.tile([P, wd], F32, name=f"tt{b}_{i}", tag="tp")
                    nc.vector.scalar_tensor_tensor(
                        tt[:],
                        xt[c][:, o : o + wd],
                        1.0,
                        mt[c][:, o : o + wd],
                        Alu.bypass,
                        Alu.mult,
                        accum_out=col(acc1, i * 3 + 1),
                    )
                    # r2 partial: sum(T^2); output value unused -> dump it
                    # into the bm tile (Sign1 overwrites it in stage 2)
                    nc.scalar.activation(
                        bm[c][:, o : o + wd],
                        tt[:],
                        Act.Square,
                        accum_out=col(acc1, i * 3 + 2),
                    )

                reduce_cols(st1[:], acc1[:], 3, len(s1_pieces), red1)
                cnt, r1, r2 = col(st1, 0), col(st1, 1), col(st1, 2)
                cntc, inv = col(sv, 0), col(sv, 1)
                mean1, nm1, e1 = col(sv, 2), col(sv, 3), col(sv, 4)
                nv1, v1c, s1 = col(sv, 5), col(sv, 6), col(sv, 7)
                tA, tB, tC, tD = col(sv, 16), col(sv, 17), col(sv, 18), col(sv, 19)
                tE, tF = col(sv, 20), col(sv, 21)

                def newton_sqrt(dst, seed, vsq, t1, t2, mid, vh):
                    # dst = sqrt(vsq), one Newton step from the ACT seed (HW
                    # Sqrt is ~7e-6 rel; one step lands ~2e-11).
                    # TT/TS only (the STT ISA struct allows one sync wait).
                    nc.vector.tensor_scalar(vh[:], vsq[:], 0.5, None, Alu.mult)
                    nc.vector.reciprocal(t1[:], seed[:])
                    nc.vector.tensor_mul(t2[:], vh[:], t1[:])
                    nc.vector.tensor_scalar(t1[:], seed[:], 0.5, None, Alu.mult)
                    nc.vector.tensor_add(dst, t1[:], t2[:])

                nc.vector.tensor_scalar(cntc, cnt, 1.0, None, Alu.max)
                nc.vector.reciprocal(inv, cntc)
                nc.vector.tensor_mul(mean1, r1, inv)
                nc.vector.tensor_scalar(nm1, mean1, -1.0, None, Alu.mult)
                nc.vector.tensor_mul(e1, r2, inv)
                nc.vector.tensor_mul(nv1, mean1, mean1)
                nc.vector.tensor_sub(tE, e1, nv1)
                nc.vector.tensor_scalar(v1c, tE, C2, 1e-30, Alu.mult, Alu.max)
                nc.scalar.activation(tC, v1c, Act.Sqrt)
                newton_sqrt(s1, tC, v1c, tA, tB, tD, tF)

                # ------------- stage 2: residual q + second-order stats ------
                for i, (c, o, wd) in enumerate(s2_pieces):
                    xs = xt[c][:, o : o + wd]
                    ms = m16[c][:, o : o + wd]
                    bs = bm[c][:, o : o + wd]
                    # sign1 straight into the bm tile, then mask in place
                    nc.scalar.activation(bs, xs, Act.Sign, bias=nm1)
                    ab = w_pool.tile([P, wd], F32, name=f"ab{b}_{i}", tag="w")
                    nc.scalar.activation(ab[:], xs, Act.Abs, bias=nm1)
                    # masked sign1 (bf16 2x, in place)
                    nc.vector.tensor_mul(bs, bs, ms)
                    # q = (|d| - s1) * b1m, overwrites the x tile; accum sum(q)
                    nc.vector.scalar_tensor_tensor(
                        xs,
                        ab[:],
                        s1,
                        bs,
                        Alu.subtract,
                        Alu.mult,
                        accum_out=col(acc2, i * 2 + 0),
                    )
                    nc.scalar.activation(
                        ab[:], xs, Act.Square, accum_out=col(acc2, i * 2 + 1)
                    )

                reduce_cols(st2[:], acc2[:], 2, len(s2_pieces), red2)
                sq, sq2 = col(st2, 0), col(st2, 1)
                mean2, nm2, e2 = col(sv, 9), col(sv, 10), col(sv, 11)
                nv2, v2c, s2, kk = col(sv, 12), col(sv, 13), col(sv, 14), col(sv, 15)

                nc.vector.tensor_mul(mean2, sq, inv)
                nc.vector.tensor_scalar(nm2, mean2, -1.0, None, Alu.mult)
                nc.vector.tensor_mul(e2, sq2, inv)
                nc.vector.tensor_mul(nv2, mean2, mean2)
                nc.vector.tensor_sub(tE, e2, nv2)
                nc.vector.tensor_scalar(v2c, tE, C2, 1e-30, Alu.mult, Alu.max)
                nc.scalar.activation(tC, v2c, Act.Sqrt)
                newton_sqrt(s2, tC, v2c, tA, tB, tD, tF)
                nc.vector.tensor_add(kk, mean1, mean2)

                # ------------- stage 3: output assembly ----------------------
                for i, (c, o, wd) in enumerate(s3_pieces):
                    qs = xt[c][:, o : o + wd]
                    ms = m16[c][:, o : o + wd]
                    bs = bm[c][:, o : o + wd]
                    b2t = b2_pool.tile([P, wd], BF16, name=f"b2_{b}_{i}", tag="b2")
                    nc.scalar.activation(b2t[:], qs, Act.Sign, bias=nm2)
                    p1 = w_pool.tile([P, wd], F32, name=f"p1_{b}_{i}", tag="w")
                    # p1 = s1*b1m + K  (TS dual-scalar, 2x)
                    nc.vector.tensor_scalar(p1[:], bs, s1, kk, Alu.mult, Alu.add)
                    # p1 += s2*b2, then *= m  (in-place, one work tile/chunk)
                    nc.vector.scalar_tensor_tensor(
                        p1[:], b2t[:], s2, p1[:], Alu.mult, Alu.add
                    )
                    nc.vector.tensor_mul(p1[:], p1[:], ms)
                    nc.sync.dma_start(
                        out[r0 : r0 + P, c * CW + o : c * CW + o + wd], p1[:]
                    )

    return nc


def get_program():
    if "nc" not in _CACHE:
        nc = _build_program()
        # Bacc defers register allocation etc. to compile()/finalize();
        # the spmd exec path serializes without finalizing.
        nc.finalize()
        _CACHE["nc"] = nc
    return _CACHE["nc"]


def kernel(x: np.ndarray, mask: np.ndarray) -> np.ndarray:
    import time

    from concourse.bass_utils import run_bass_kernel_spmd

    x = np.ascontiguousarray(np.asarray(x, dtype=np.float32))
    mask = np.ascontiguousarray(np.asarray(mask))
    if mask.dtype == np.bool_ or mask.dtype == np.uint8:
        mask_u8 = mask.view(np.uint8)
    else:
        mask_u8 = (mask != 0).astype(np.uint8)
    assert x.shape == (R * NCORES, N), x.shape
    assert mask_u8.shape == (R * NCORES, N), mask_u8.shape

    nc = get_program()
    in_maps = [
        {
            "x": x[k * R : (k + 1) * R],
            "mask": mask_u8[k * R : (k + 1) * R],
        }
        for k in range(NCORES)
    ]
    last_err = None
    for attempt in range(3):
        try:
            res = run_bass_kernel_spmd(nc, in_maps, core_ids=list(range(NCORES)))
            return np.concatenate([r["out"] for r in res.results], axis=0)
        except Exception as e:  # transient NRT/device hiccups
            last_err = e
            if attempt < 2:
                time.sleep(10)
    raise last_err


if __name__ == "__main__":
    xs = np.random.randn(R * NCORES, N).astype(np.float32)
    ms = (np.random.randint(0, 2, (R * NCORES, N))).astype(bool)
    y = kernel(xs, ms)
    print(y.shape, y.dtype)


# revision 54
# speedup vs baseline: 1.3397x; 1.3397x over previous
"""Trainium2 Bass kernel for adaptive_high_order_residual_v2 (ORDER=2 masked
sign-binarization, per-row stats).

Full-input contract: kernel(x, mask) takes the complete (4096, 11008) arrays,
shards rows across 8 NeuronCores (512 rows each; per-row reductions make this
embarrassingly parallel), runs one SPMD Bass program, and concatenates the
per-core outputs.

Math per row (ORDER = 2, exact restructuring of the reference):
  t    = x * m                      (masked input)
  mean1 = sum(t)/cnt ; var1 = sum(t^2)/cnt - mean1^2 ; s1 = sqrt(var1 * 2/pi)
  b1   = sign(x - mean1)            (valid entries only; invalid masked later)
  q    = (|x - mean1| - s1) * b1*m  (== residual_2 of the reference: d - s1*b1)
  mean2 = sum(q)/cnt ; var2 = sum(q^2)/cnt - mean2^2 ; s2 = sqrt(var2 * 2/pi)
  b2   = sign(q - mean2)
  out  = ((mean1 + mean2) + s1*b1 + s2*b2) * m

Engine split per 128x2752 chunk:
  ACT: mask cast (+cnt accum), square(T) (+r2), Sign->b1, Abs->|d|,
       square(q) (+sum q^2), Sign->b2
  DVE: T=x*m (+r1 accum), b1m=b1*m16 (bf16 2x), q=(|d|-s1)*b1m (+sum q),
       p1=s1*b1m+K (2x), p2=s2*b2+p1, out=p2*m
"""

import sys

import numpy as np

sys.path.insert(0, "/opt/trn_rl_repo")

R = 512          # rows per core
N = 11008        # columns
P = 128          # SBUF partitions per row-block
NBLK = R // P    # 4 blocks per core
CW = 2752        # column chunk width
NCH = N // CW    # 4 chunks per block
NCORES = 8
C2 = 0.6366197723675814  # 2/pi

_CACHE = {}


def _build_program():
    import concourse.bacc as bacc
    import concourse.mybir as mybir
    from concourse.tile import TileContext

    F32 = mybir.dt.float32
    BF16 = mybir.dt.bfloat16
    U8 = mybir.dt.uint8
    Alu = mybir.AluOpType
    Act = mybir.ActivationFunctionType

    nc = bacc.Bacc()
    x = nc.dram_tensor("x", [R, N], F32, kind="ExternalInput")
    mk = nc.dram_tensor("mask", [R, N], U8, kind="ExternalInput")
    out = nc.dram_tensor("out", [R, N], F32, kind="ExternalOutput")

    with TileContext(nc) as tc:
        with (
            tc.tile_pool(name="xq", bufs=6) as xq_pool,    # x tile, later holds q
            tc.tile_pool(name="m8", bufs=2) as m8_pool,    # raw u8 mask (cast only)
            tc.tile_pool(name="m16", bufs=5) as m16_pool,  # bf16 mask
            tc.tile_pool(name="bmp", bufs=5) as bm_pool,   # masked sign1 (bf16)
            tc.tile_pool(name="b2p", bufs=1) as b2_pool,   # sign2 + ACT garbage
            tc.tile_pool(name="w", bufs=4) as w_pool,      # f32 rotating work
            tc.tile_pool(name="tp", bufs=3) as t_pool,     # T tiles (stage 1)
            tc.tile_pool(name="sc", bufs=2) as sc_pool,    # scalars + accums
        ):
            for b in range(NBLK):
                r0 = b * P

                xt = [
                    xq_pool.tile([P, CW], F32, name=f"xt{b}_{c}", tag="xq")
                    for c in range(NCH)
                ]
                mt = [
                    m8_pool.tile([P, CW], U8, name=f"mt{b}_{c}", tag="m8")
                    for c in range(NCH)
                ]
                m16 = [
                    m16_pool.tile([P, CW], BF16, name=f"m16_{b}_{c}", tag="m16")
                    for c in range(NCH)
                ]
                bm = [
                    bm_pool.tile([P, CW], BF16, name=f"bm{b}_{c}", tag="bm")
                    for c in range(NCH)
                ]
                # accumulators, chunk-major interleave: col = c*nq + q so the
                # pairwise tree reduce uses contiguous 2-D slices
                acc1 = sc_pool.tile([P, 3 * 2 * NCH], F32, name=f"acc1_{b}", tag="acc1")
                acc2 = sc_pool.tile([P, 2 * (NCH + 1)], F32, name=f"acc2_{b}", tag="acc2")
                st1 = sc_pool.tile([P, 3], F32, name=f"st1_{b}", tag="st1")
                st2 = sc_pool.tile([P, 2], F32, name=f"st2_{b}", tag="st2")
                red1 = sc_pool.tile([P, 18], F32, name=f"red1_{b}", tag="red1")
                red2 = sc_pool.tile([P, 6], F32, name=f"red2_{b}", tag="red2")
                sv = sc_pool.tile([P, 24], F32, name=f"sv_{b}", tag="sv")

                def col(t, i):
                    return t[:, i : i + 1]

                # piece lists: (chunk, col offset, width). The first chunk of
                # each stage is split in half so the consuming engine starts
                # ~1.2us sooner after a stage boundary; block 0 also splits
                # the very first DMA for a faster ramp.
                H = CW // 2

                def pieces_for(split_first, split_last=False):
                    ps = []
                    for c in range(NCH):
                        if (c == 0 and split_first) or (
                            c == NCH - 1 and split_last
                        ):
                            ps.append((c, 0, H))
                            ps.append((c, H, H))
                        else:
                            ps.append((c, 0, CW))
                    return ps

                s1_pieces = pieces_for(False)
                s2_pieces = pieces_for(False)
                s3_pieces = pieces_for(False, split_last=(b == NBLK - 1))

                def reduce_cols(dst, acc, nq, npieces, red):
                    # sum piece-major accum columns: col = p*nq + q
                    if npieces == 8:
                        nc.vector.tensor_add(
                            red[:, 0 : 4 * nq], acc[:, 0 : 4 * nq],
                            acc[:, 4 * nq : 8 * nq],
                        )
                        nc.vector.tensor_add(
                            red[:, 4 * nq : 6 * nq], red[:, 0 : 2 * nq],
                            red[:, 2 * nq : 4 * nq],
                        )
                        nc.vector.tensor_add(
                            dst, red[:, 4 * nq : 5 * nq], red[:, 5 * nq : 6 * nq]
                        )
                    elif npieces == 4:
                        nc.vector.tensor_add(
                            red[:, 0 : 2 * nq], acc[:, 0 : 2 * nq],
                            acc[:, 2 * nq : 4 * nq],
                        )
                        nc.vector.tensor_add(dst, red[:, 0:nq], red[:, nq : 2 * nq])
                    elif npieces == 5:
                        nc.vector.tensor_add(
                            red[:, 0 : 2 * nq], acc[:, 0 : 2 * nq],
                            acc[:, 2 * nq : 4 * nq],
                        )
                        nc.vector.tensor_add(
                            red[:, 2 * nq : 3 * nq], red[:, 0:nq], red[:, nq : 2 * nq]
                        )
                        nc.vector.tensor_add(
                            dst, red[:, 2 * nq : 3 * nq], acc[:, 4 * nq : 5 * nq]
                        )
                    else:
                        raise AssertionError(npieces)

                # ------------- stage 1: masked first-order stats -------------
                for i, (c, o, wd) in enumerate(s1_pieces):
                    if o == 0:
                        nc.sync.dma_start(
                            xt[c][:, 0:wd], x[r0 : r0 + P, c * CW : c * CW + wd]
                        )
                        nc.sync.dma_start(
                            mt[c][:, 0:wd], mk[r0 : r0 + P, c * CW : c * CW + wd]
                        )
                    else:
                        nc.sync.dma_start(
                            xt[c][:, o : o + wd],
                            x[r0 : r0 + P, c * CW + o : c * CW + o + wd],
                        )
                        nc.sync.dma_start(
                            mt[c][:, o : o + wd],
                            mk[r0 : r0 + P, c * CW + o : c * CW + o + wd],
                        )
                    # mask cast to bf16 + cnt partial
                    nc.scalar.activation(
                        m16[c][:, o : o + wd],
                        mt[c][:, o : o + wd],
                        Act.Copy,
                        accum_out=col(acc1, i * 3 + 0),
                    )
                    # T = x*m + r1 partial
                    tt = t_pool.tile([P, wd], F32, name=f"tt{b}_{i}", tag="tp")
                    nc.vector.scalar_tensor_tensor(
                        tt[:],
                        xt[c][:, o : o + wd],
                        1.0,
                        mt[c][:, o : o + wd],
                        Alu.bypass,
                        Alu.mult,
                        accum_out=col(acc1, i * 3 + 1),
                    )
                    # r2 partial: sum(T^2); output value unused -> dump it
                    # into the bm tile (Sign1 overwrites it in stage 2)
                    nc.scalar.activation(
                        bm[c][:, o : o + wd],
                        tt[:],
                        Act.Square,
                        accum_out=col(acc1, i * 3 + 2),
                    )

                reduce_cols(st1[:], acc1[:], 3, len(s1_pieces), red1)
                cnt, r1, r2 = col(st1, 0), col(st1, 1), col(st1, 2)
                cntc, inv = col(sv, 0), col(sv, 1)
                mean1, nm1, e1 = col(sv, 2), col(sv, 3), col(sv, 4)
                nv1, v1c, s1 = col(sv, 5), col(sv, 6), col(sv, 7)
                tA, tB, tC, tD = col(sv, 16), col(sv, 17), col(sv, 18), col(sv, 19)
                tE, tF = col(sv, 20), col(sv, 21)

                def newton_sqrt(dst, seed, vsq, t1, t2, mid, vh):
                    # dst = sqrt(vsq), one Newton step from the ACT seed (HW
                    # Sqrt is ~7e-6 rel; one step lands ~2e-11).
                    # TT/TS only (the STT ISA struct allows one sync wait).
                    nc.vector.tensor_scalar(vh[:], vsq[:], 0.5, None, Alu.mult)
                    nc.vector.reciprocal(t1[:], seed[:])
                    nc.vector.tensor_mul(t2[:], vh[:], t1[:])
                    nc.vector.tensor_scalar(t1[:], seed[:], 0.5, None, Alu.mult)
                    nc.vector.tensor_add(dst, t1[:], t2[:])

                nc.vector.tensor_scalar(cntc, cnt, 1.0, None, Alu.max)
                nc.vector.reciprocal(inv, cntc)
                nc.vector.tensor_mul(mean1, r1, inv)
                nc.vector.tensor_scalar(nm1, mean1, -1.0, None, Alu.mult)
                nc.vector.tensor_mul(e1, r2, inv)
                nc.vector.tensor_mul(nv1, mean1, mean1)
                nc.vector.tensor_sub(tE, e1, nv1)
                nc.vector.tensor_scalar(v1c, tE, C2, 1e-30, Alu.mult, Alu.max)
                nc.scalar.activation(tC, v1c, Act.Sqrt)
                newton_sqrt(s1, tC, v1c, tA, tB, tD, tF)

                # ------------- stage 2: residual q + second-order stats ------
                for i, (c, o, wd) in enumerate(s2_pieces):
                    xs = xt[c][:, o : o + wd]
                    ms = m16[c][:, o : o + wd]
                    bs = bm[c][:, o : o + wd]
                    # sign1 straight into the bm tile, then mask in place
                    nc.scalar.activation(bs, xs, Act.Sign, bias=nm1)
                    ab = w_pool.tile([P, wd], F32, name=f"ab{b}_{i}", tag="w")
                    nc.scalar.activation(ab[:], xs, Act.Abs, bias=nm1)
                    # masked sign1 (bf16 2x, in place)
                    nc.vector.tensor_mul(bs, bs, ms)
                    # q = (|d| - s1) * b1m, overwrites the x tile; accum sum(q)
                    nc.vector.scalar_tensor_tensor(
                        xs,
                        ab[:],
                        s1,
                        bs,
                        Alu.subtract,
                        Alu.mult,
                        accum_out=col(acc2, i * 2 + 0),
                    )
                    nc.scalar.activation(
                        ab[:], xs, Act.Square, accum_out=col(acc2, i * 2 + 1)
                    )

                reduce_cols(st2[:], acc2[:], 2, len(s2_pieces), red2)
                sq, sq2 = col(st2, 0), col(st2, 1)
                mean2, nm2, e2 = col(sv, 9), col(sv, 10), col(sv, 11)
                nv2, v2c, s2, kk = col(sv, 12), col(sv, 13), col(sv, 14), col(sv, 15)

                nc.vector.tensor_mul(mean2, sq, inv)
                nc.vector.tensor_scalar(nm2, mean2, -1.0, None, Alu.mult)
                nc.vector.tensor_mul(e2, sq2, inv)
                nc.vector.tensor_mul(nv2, mean2, mean2)
                nc.vector.tensor_sub(tE, e2, nv2)
                nc.vector.tensor_scalar(v2c, tE, C2, 1e-30, Alu.mult, Alu.max)
                nc.scalar.activation(tC, v2c, Act.Sqrt)
                newton_sqrt(s2, tC, v2c, tA, tB, tD, tF)
                nc.vector.tensor_add(kk, mean1, mean2)

                # ------------- stage 3: output assembly ----------------------
                for i, (c, o, wd) in enumerate(s3_pieces):
                    qs = xt[c][:, o : o + wd]
                    ms = m16[c][:, o : o + wd]
                    bs = bm[c][:, o : o + wd]
                    b2t = b2_pool.tile([P, wd], BF16, name=f"b2_{b}_{i}", tag="b2")
                    nc.scalar.activation(b2t[:], qs, Act.Sign, bias=nm2)
                    p1 = w_pool.tile([P, wd], F32, name=f"p1_{b}_{i}", tag="w")
                    # p1 = s1*b1m + K  (TS dual-scalar, 2x)
                    nc.vector.tensor_scalar(p1[:], bs, s1, kk, Alu.mult, Alu.add)
                    # p1 += s2*b2, then *= m  (in-place, one work tile/chunk)
                    nc.vector.scalar_tensor_tensor(
                        p1[:], b2t[:], s2, p1[:], Alu.mult, Alu.add
                    )
                    nc.vector.tensor_mul(p1[:], p1[:], ms)
                    nc.sync.dma_start(
                        out[r0 : r0 + P, c * CW + o : c * CW + o + wd], p1[:]
                    )

    return nc


def get_program():
    if "nc" not in _CACHE:
        nc = _build_program()
        # Bacc defers register allocation etc. to compile()/finalize();
        # the spmd exec path serializes without finalizing.
        nc.finalize()
        _CACHE["nc"] = nc
    return _CACHE["nc"]


def kernel(x: np.ndarray, mask: np.ndarray) -> np.ndarray:
    import time

    from concourse.bass_utils import run_bass_kernel_spmd

    x = np.ascontiguousarray(np.asarray(x, dtype=np.float32))
    mask = np.ascontiguousarray(np.asarray(mask))
    if mask.dtype == np.bool_ or mask.dtype == np.uint8:
        mask_u8 = mask.view(np.uint8)
    else:
        mask_u8 = (mask != 0).astype(np.uint8)
    assert x.shape == (R * NCORES, N), x.shape
    assert mask_u8.shape == (R * NCORES, N), mask_u8.shape

    nc = get_program()
    in_maps = [
        {
            "x": x[k * R : (k + 1) * R],
            "mask": mask_u8[k * R : (k + 1) * R],
        }
        for k in range(NCORES)
    ]
    last_err = None
    for attempt in range(3):
        try:
            res = run_bass_kernel_spmd(nc, in_maps, core_ids=list(range(NCORES)))
            return np.concatenate([r["out"] for r in res.results], axis=0)
        except Exception as e:  # transient NRT/device hiccups
            last_err = e
            if attempt < 2:
                time.sleep(10)
    raise last_err


if __name__ == "__main__":
    xs = np.random.randn(R * NCORES, N).astype(np.float32)
    ms = (np.random.randint(0, 2, (R * NCORES, N))).astype(bool)
    y = kernel(xs, ms)
    print(y.shape, y.dtype)
